# revision 9
# baseline (speedup 1.0000x reference)
"""Trainium2 Bass kernel for an Elman RNN encoder (embedding -> input
projection GEMM -> 512-step tanh recurrence).

Shapes (hardcoded): T=512, B=64, V=50257, E=300, H=512.
Sharding: data-parallel over batch across 8 NeuronCores (8 batch each);
embedding table and RNN weights replicated.

Per-core layout choices:
  - Embedding rows are gathered via indirect DMA (tokens for this core's
    batch slice, row order r = t*8 + b), PE-transposed to put the embedding
    dim on partitions, then a fp32 GEMM against W_ih^T produces the input
    projection xp directly in "hidden-major" layout:
        xp_sb[p, t*32 + m*8 + b] = xp[t, b, m*128 + p]
    The bias (b_ih + b_hh) is folded in by appending a constant-1 column to
    the embedding table and a bias row to W_ih^T.
  - Recurrence: out[h_new, b] = W_hhT_tile.T @ h, with W_hh^T in bf16 as
    16 stationary [128,128] tiles (FWL halves the weight-load cost) and the
    bf16 hidden state as the 8-wide moving operand. The tanh output layout
    is directly the next step's moving operand - no per-step transposes.
  - Outputs are written fp32 (tanh computed from fp32 PSUM), staged
    hidden-major, transposed to batch-major in 32x32 blocks on the vector
    engine off the critical path, and DMA'd out in 2KB-contiguous runs.
"""

import sys

for _p in ("/opt/trn_rl_repo",):
    if _p not in sys.path:
        sys.path.insert(0, _p)

from contextlib import ExitStack

import ml_dtypes
import numpy as np

import concourse.bacc as bacc
import concourse.bass as bass
import concourse.tile as tile
from concourse import mybir
from concourse.bass_utils import run_bass_kernel_spmd

T, B, V, E, H = 512, 64, 50257, 300, 512
NCORES = 8
BL = B // NCORES            # batch per core = 8
EP = 304                    # embedding dim padded: 300 + bias-one col + 3 zeros
KCH = [(0, 128), (128, 128), (256, 48)]   # K chunks over EP
R = T * BL                  # gathered rows per core = 4096
NRC = 8                     # row chunks of 512 for the GEMM
NM = H // 128               # 4 hidden tiles

F32 = mybir.dt.float32
BF16 = mybir.dt.bfloat16
I32 = mybir.dt.int32
TANH = mybir.ActivationFunctionType.Tanh

_compiled = {}


def _build():
    nc = bacc.Bacc()
    tbl_d = nc.declare_dram_parameter("tbl", [V, EP], F32, isOutput=False)
    idx_d = nc.declare_dram_parameter("idx", [128, R // 128], I32, isOutput=False)
    wih_d = nc.declare_dram_parameter("wih", [128, 3 * H], F32, isOutput=False)
    whh_d = nc.declare_dram_parameter("whh", [128, 4 * H], BF16, isOutput=False)
    ident_d = nc.declare_dram_parameter("ident", [128, 128], F32, isOutput=False)
    out_d = nc.declare_dram_parameter("out", [T, BL, H], F32, isOutput=True)

    with tile.TileContext(nc) as tc:
        with ExitStack() as ctx:
            consts = ctx.enter_context(tc.tile_pool(name="consts", bufs=1))
            wih_sb = consts.tile([128, 3 * H], F32)
            whh_sb = consts.tile([128, 4 * H], BF16)
            idx_sb = consts.tile([128, R // 128], I32)
            id_sb = consts.tile([128, 128], F32)
            xp_sb = consts.tile([128, T * 4 * BL], F32)   # 64KB/partition

            nc.sync.dma_start(wih_sb[:], wih_d[:])
            nc.sync.dma_start(whh_sb[:], whh_d[:])
            nc.sync.dma_start(idx_sb[:], idx_d[:])
            nc.sync.dma_start(id_sb[:], ident_d[:])

            # ---- Phase 1: gather + transpose + input-projection GEMM ----
            with tc.tile_pool(name="erow", bufs=3) as e_pool, \
                 tc.tile_pool(name="eT", bufs=2) as eT_pool, \
                 tc.tile_pool(name="tp", bufs=3, space="PSUM") as tp_pool, \
                 tc.tile_pool(name="gp", bufs=4, space="PSUM") as gp_pool:
                for rc in range(NRC):
                    eT = eT_pool.tile([128, 3 * 512], F32)
                    for j in range(4):
                        et = e_pool.tile([128, EP], F32)
                        nc.gpsimd.indirect_dma_start(
                            out=et[:],
                            out_offset=None,
                            in_=tbl_d[:],
                            in_offset=bass.IndirectOffsetOnAxis(
                                ap=idx_sb[:, rc * 4 + j: rc * 4 + j + 1], axis=0
                            ),
                        )
                        for k, (k0, kl) in enumerate(KCH):
                            tp = tp_pool.tile([128, 128], F32)
                            nc.tensor.transpose(
                                tp[:kl, :], et[:, k0: k0 + kl], id_sb[:]
                            )
                            nc.vector.tensor_copy(
                                eT[:kl, k * 512 + j * 128: k * 512 + (j + 1) * 128],
                                tp[:kl, :],
                            )
                    for m in range(NM):
                        gp = gp_pool.tile([128, 512], F32)
                        for k, (k0, kl) in enumerate(KCH):
                            nc.tensor.matmul(
                                gp[:],
                                lhsT=wih_sb[:kl, k * H + m * 128: k * H + (m + 1) * 128],
                                rhs=eT[:kl, k * 512: (k + 1) * 512],
                                start=(k == 0),
                                stop=(k == len(KCH) - 1),
                            )
                        # xp_sb[p, (rc*64+t)*32 + m*8 + b] = gp[p, t*8+b]
                        dst = (
                            xp_sb[:, rc * 2048: (rc + 1) * 2048]
                            .rearrange("p (t g) -> p t g", g=32)[:, :, m * BL: (m + 1) * BL]
                        )
                        nc.scalar.copy(dst, gp[:].rearrange("p (t b) -> p t b", b=BL))

            # ---- Phase 2: recurrence ----
            # Per step: one fp32 identity-matmul seeds PSUM with xp (start=True),
            # then 16 bf16 W_hh^T matmuls accumulate onto it; a single ACT tanh
            # produces the bf16 h for the next step (critical path MM->tanh->MM)
            # and a second ACT tanh writes the fp32 outs staging off-path.
            with tc.tile_pool(name="h", bufs=3) as h_pool, \
                 tc.tile_pool(name="stage", bufs=2) as stage_pool, \
                 tc.tile_pool(name="dmab", bufs=2) as dmab_pool, \
                 tc.tile_pool(name="rp", bufs=4, space="PSUM") as rp_pool:
                h_prev = h_pool.tile([128, 4 * BL], BF16)
                nc.vector.memset(h_prev[:], 0)
                stage = None
                dmab = None
                for t in range(T):
                    ts_ = t % 4
                    if ts_ == 0:
                        stage = stage_pool.tile([128, 4 * 4 * BL], F32)
                    if t % 32 == 0:
                        dmab = dmab_pool.tile([32, 8 * H], F32)
                    g32 = (t % 32) // 4
                    rp = rp_pool.tile([128, 4 * BL], F32)
                    nc.tensor.matmul(
                        rp[:], lhsT=id_sb[:],
                        rhs=xp_sb[:, t * 32: (t + 1) * 32],
                        start=True, stop=False, skip_group_check=True,
                    )
                    for m in range(NM):
                        for k in range(4):
                            nc.tensor.matmul(
                                rp[:, m * BL: (m + 1) * BL],
                                lhsT=whh_sb[:, k * H + m * 128: k * H + m * 128 + 128],
                                rhs=h_prev[:, k * BL: (k + 1) * BL],
                                start=False,
                                stop=(m == NM - 1 and k == 3),
                                skip_group_check=True,
                            )
                    h_new = h_pool.tile([128, 4 * BL], BF16)
                    nc.scalar.activation(h_new[:], rp[:], TANH)
                    st_out = stage[:].rearrange("p (m s) -> p m s", s=4 * BL)[
                        :, :, ts_ * BL: (ts_ + 1) * BL
                    ]
                    nc.scalar.activation(
                        st_out, rp[:].rearrange("p (m b) -> p m b", b=BL), TANH
                    )
                    h_prev = h_new
                    if ts_ == 3:
                        # stage [128, 4m*32(ts,b)] -> dmab [32(ts,b), g32*512 + h]
                        for m in range(NM):
                            for q in range(4):
                                nc.vector.transpose(
                                    dmab[:, g32 * 512 + m * 128 + q * 32:
                                         g32 * 512 + m * 128 + (q + 1) * 32],
                                    stage[q * 32: (q + 1) * 32, m * 32: (m + 1) * 32],
                                )
                    if t % 32 == 31:
                        # partition-split rearranges silently break DMA APs;
                        # use plain 2D slices, one DMA per timestep
                        t0 = t - 31
                        for g2 in range(8):
                            for s2 in range(4):
                                nc.sync.dma_start(
                                    out_d[t0 + g2 * 4 + s2],
                                    dmab[s2 * BL: (s2 + 1) * BL,
                                         g2 * H: (g2 + 1) * H],
                                )
    nc.finalize()
    return nc


def _get_compiled():
    if "nc" not in _compiled:
        _compiled["nc"] = _build()
    return _compiled["nc"]


def kernel(src, emb, W_ih, W_hh, b_ih, b_hh):
    src = np.asarray(src)
    emb = np.asarray(emb, dtype=np.float32)
    W_ih = np.asarray(W_ih, dtype=np.float32)
    W_hh = np.asarray(W_hh, dtype=np.float32)
    b_ih = np.asarray(b_ih, dtype=np.float32)
    b_hh = np.asarray(b_hh, dtype=np.float32)

    nc = _get_compiled()

    # Embedding table padded with a constant-1 column (bias trick) + zeros.
    tbl = np.zeros((V, EP), dtype=np.float32)
    tbl[:, :E] = emb
    tbl[:, E] = 1.0

    # W_ih^T padded: rows 0..299 = W_ih.T, row 300 = b_ih + b_hh, rest 0;
    # packed as [128, 3*H] with K-chunk k at cols [k*H, (k+1)*H).
    wihT = np.zeros((3 * 128, H), dtype=np.float32)
    wihT[:E] = W_ih.T
    wihT[E] = b_ih + b_hh
    wih_host = np.ascontiguousarray(
        wihT.reshape(3, 128, H).transpose(1, 0, 2).reshape(128, 3 * H)
    )

    # W_hh^T in bf16 packed as [128, 4*H]: whh[p, k*H + j] = W_hh[j, k*128+p]
    whhT = W_hh.T.astype(ml_dtypes.bfloat16)      # [h_old, h_new]
    whh_host = np.ascontiguousarray(
        whhT.reshape(4, 128, H).transpose(1, 0, 2).reshape(128, 4 * H)
    )

    ident = np.eye(128, dtype=np.float32)

    # Token indices per core: idx[p, j] = src[t, c*8 + b], r = j*128+p = t*8+b
    src32 = src.astype(np.int32)
    in_maps = []
    for c in range(NCORES):
        toks = src32[:, c * BL: (c + 1) * BL].reshape(R)   # r = t*8 + b
        idx_host = np.ascontiguousarray(toks.reshape(R // 128, 128).T)
        in_maps.append({
            "tbl": tbl,
            "idx": idx_host,
            "wih": wih_host,
            "whh": whh_host,
            "ident": ident,
        })

    res = run_bass_kernel_spmd(nc, in_maps, list(range(NCORES)))
    _compiled["last_res"] = res

    outs = np.empty((T, B, H), dtype=np.float32)
    for c in range(NCORES):
        outs[:, c * BL: (c + 1) * BL, :] = res.results[c]["out"]
    h_last = outs[-1][None]
    return outs, h_last
